# Initial kernel scaffold
#
"""2-layer GIN + attentional pooling on 8 Trainium2 NeuronCores (Bass/Tile).

Architecture (no usable DMA scatter/gather on this runtime):
  - Nodes split into 8 graph-aligned ownership ranges (one per core); each
    core processes edges whose dst it owns.
  - Per core, edges are bucketed by fixed 32768-node src blocks (one per
    GPSIMD core-group) and ordered by dst within uniform node-chunks.
  - x / h1 live in SBUF as per-block gather tables; GPSIMD ap_gather fetches
    per-edge source rows (feature-major).
  - Segment sums over dst: custom DVE prefix-scan over the dst-sorted edge
    stream, ap_gather of the cumsum at host-known segment-end positions,
    shifted subtraction. Group partials fold via a block-ones PE matmul.
  - h1 exchanged across cores with AllGather.
  - Pooling: constant-shift softmax (exp(g-20); gate range bounded for this
    input scale) + the same cumsum trick over graph segments.
"""
import os
import sys

os.environ.setdefault("NEURON_RT_RESET_CORES", "1")
sys.path.insert(0, '/opt/trn_rl_repo')

import numpy as np

# -- NTFF profiling hook shim (optional; enables trace=True under axon) ----
def _install_ntff_shim():
    import types
    try:
        import antenv
        if 'antenv.axon_hooks' in sys.modules:
            return
        hooks = types.ModuleType('antenv.axon_hooks')
        _state = {'hook': None}
        hooks.set_axon_ntff_profile_hook = lambda h: _state.__setitem__('hook', h)
        hooks.get_axon_ntff_profile_hook = lambda: _state['hook']
        sys.modules['antenv.axon_hooks'] = hooks
        antenv.axon_hooks = hooks
        from trn_agent_boot.trn_boot import _ntff_profile_via_ctypes
        h = _ntff_profile_via_ctypes('/opt/axon/libaxon_pjrt.so')
        if h is not None:
            hooks.set_axon_ntff_profile_hook(h)
    except Exception:
        pass


_install_ntff_shim()

N_NODES = 262144
N_GRAPHS = 1024
C_IN = 16
H = 32
NC = 8
BLK = 32768
NCH1, ECH1, NCHUNK1 = 2112, 4608, 16
NCH2, ECH2, NCHUNK2 = 1056, 2432, 32
NMAX = NCH1 * NCHUNK1            # 33792
NCH3 = NMAX // 4                 # 8448
GPAD = 64
GW = 4 * GPAD                    # 256 graph slots
SOFTMAX_SHIFT = 20.0
MAX_WAITS = 1
TILE_N = 512

_cache = {}


def _split_multi_waits(nc, mybir, max_waits=MAX_WAITS):
    n_split = 0
    for fn in nc.m.functions:
        for bb in fn.blocks:
            out = []
            for ins in bb.instructions:
                si = ins.sync_info
                if si is not None and si.on_wait and len(si.on_wait) > max_waits:
                    waits = list(si.on_wait)
                    extra = waits[:-max_waits]
                    keep = waits[-max_waits:]
                    for i in range(0, len(extra), max_waits):
                        group = extra[i:i + max_waits]
                        nop = mybir.InstNoOp(
                            name=f"waitsplit_{nc.next_id()}",
                            sync_info=mybir.SyncInfo(on_wait=group, on_update=[]),
                            bass_nofuse=True,
                            engine=ins.engine,
                        )
                        out.append(nop)
                        n_split += 1
                    si.on_wait = keep
                out.append(ins)
            bb.instructions = out
    return n_split


def _wrap_idx(vals, group, arr, col0=0):
    """Wrapped ap_gather index layout: value i -> arr[16g + i%16, col0 + i//16]."""
    n = len(vals)
    assert n % 16 == 0
    v = np.asarray(vals, dtype=np.int16).reshape(n // 16, 16).T
    arr[16 * group:16 * group + 16, col0:col0 + n // 16] = v


def _register_cumsum():
    from concourse import dve_ops
    from concourse.dve_spec import Spec, Src0, C0, AluOp, lower
    import concourse.dve_spec as ds
    from concourse.dve_uop import DveOpSpec
    for op in dve_ops.OPS:
        if op.name == "CUMSUM_ANT":
            return op
    spec = Spec(
        body=ds.scan(AluOp.ADD, Src0, init=C0),
        reference=lambda in0, s0: np.cumsum(in0.astype(np.float32), axis=-1) + s0,
    )
    shas = {}
    for ver in ("v3", "v4"):
        uops = lower(spec, ver=ver)
        shas[ver] = DveOpSpec(name="CUMSUM_ANT", opcode=1, uops=uops,
                              rd1_en=False).sha(ver)
    op = dve_ops.DveOp("CUMSUM_ANT", spec, subdim=False, uops_sha=shas)
    dve_ops.OPS.append(op)
    dve_ops.CUSTOM_DVE_SPECS["CUMSUM_ANT"] = spec
    dve_ops._SUB_OPCODE_FOR_NAME["CUMSUM_ANT"] = \
        max(dve_ops._SUB_OPCODE_FOR_NAME.values()) + 1
    return op


# ================================================================ host prep
def _prep(edge_index, batch_vec):
    src = np.asarray(edge_index[0], dtype=np.int64)
    dst = np.asarray(edge_index[1], dtype=np.int64)
    bv = np.asarray(batch_vec, dtype=np.int64)

    gstart = np.searchsorted(bv, np.arange(N_GRAPHS))
    bounds = [0]
    for c in range(1, NC):
        target = c * (N_NODES // NC)
        gi = np.searchsorted(gstart, target)
        cand = []
        if gi < N_GRAPHS:
            cand.append(int(gstart[gi]))
        if gi > 0:
            cand.append(int(gstart[gi - 1]))
        bounds.append(min(cand, key=lambda v: abs(v - target)))
    bounds.append(N_NODES)
    n_lo = np.array(bounds[:-1])
    n_hi = np.array(bounds[1:])
    sizes = n_hi - n_lo
    assert sizes.max() <= NMAX, sizes
    g_lo = np.searchsorted(gstart, n_lo)
    g_hi = np.searchsorted(gstart, n_hi)

    owner = np.searchsorted(n_hi, dst, side='right')

    cores = []
    for c in range(NC):
        m = owner == c
        csrc = src[m]
        cdst_local = dst[m] - n_lo[c]
        size_c = int(sizes[c])

        ge1 = np.zeros((128, NCHUNK1 * ECH1 // 16), np.int16)
        gd1 = np.zeros((128, NCHUNK1 * NCH1 // 16), np.int16)
        ge2 = np.zeros((128, NCHUNK2 * ECH2 // 16), np.int16)
        gd2 = np.zeros((128, NCHUNK2 * NCH2 // 16), np.int16)

        blk_of = csrc >> 15
        src_local_all = (csrc & (BLK - 1))

        for k in range(NC):
            bm = blk_of == k
            bsrc = src_local_all[bm]
            bdst = cdst_local[bm]
            order = np.argsort(bdst, kind='stable')
            bsrc = bsrc[order].astype(np.int16)
            bdst = bdst[order]
            cnt = np.bincount(bdst, minlength=NMAX)
            cum = np.concatenate([[0], np.cumsum(cnt)])

            for (nch, ech, nchunks, ge, gd) in (
                    (NCH1, ECH1, NCHUNK1, ge1, gd1),
                    (NCH2, ECH2, NCHUNK2, ge2, gd2)):
                for ch in range(nchunks):
                    a, b = ch * nch, (ch + 1) * nch
                    e0, e1 = cum[a], cum[b]
                    ne = int(e1 - e0)
                    assert ne <= ech, (c, k, ch, ne, ech)
                    ev = np.zeros(ech, np.int16)
                    ev[:ne] = bsrc[e0:e1]
                    _wrap_idx(ev, k, ge, col0=ch * ech // 16)
                    ends = (cum[a + 1:b + 1] - e0).astype(np.int16)
                    _wrap_idx(ends, k, gd, col0=ch * nch // 16)

        # pooling graph ends
        if g_hi[c] < N_GRAPHS:
            gend = gstart[g_lo[c] + 1:g_hi[c] + 1].astype(np.int64)
        else:
            gend = np.concatenate([gstart[g_lo[c] + 1:], [N_NODES]]).astype(np.int64)
        gend = gend - n_lo[c]
        gp = np.zeros((32, 4 * GPAD // 16), np.int16)
        slot_map = np.full(GW, -1, np.int64)
        for ch in range(4):
            a, b = ch * NCH3, (ch + 1) * NCH3
            sel = np.where((gend > a) & (gend <= b))[0]
            assert len(sel) <= GPAD, len(sel)
            ev = np.zeros(GPAD, np.int16)
            ev[:len(sel)] = (gend[sel] - a).astype(np.int16)
            if 0 < len(sel) < GPAD:
                ev[len(sel):] = ev[len(sel) - 1]
            # (empty chunk -> idx 0 -> cs3 col0 which holds the carry)
            for j, g in enumerate(sel):
                slot_map[ch * GPAD + j] = int(g)
            for grp in range(2):
                _wrap_idx(ev, grp, gp, col0=ch * GPAD // 16)

        cores.append(dict(
            n_lo=int(n_lo[c]), size=size_c, g_lo=int(g_lo[c]), g_hi=int(g_hi[c]),
            ge1=ge1, gd1=gd1, ge2=ge2, gd2=gd2, gp=gp, slot_map=slot_map,
        ))
    return cores, [int(b) for b in bounds]


# ================================================================ device
def _build_program(bounds):
    from concourse import bacc, tile
    from concourse.bass import mybir

    CUMSUM = _register_cumsum()

    f32 = mybir.dt.float32
    f16 = mybir.dt.float16
    i16 = mybir.dt.int16
    RELU = mybir.ActivationFunctionType.Relu
    EXP = mybir.ActivationFunctionType.Exp
    ADD = mybir.AluOpType.add
    SUB = mybir.AluOpType.subtract
    MUL = mybir.AluOpType.mult

    nc = bacc.Bacc("TRN2", target_bir_lowering=False, debug=False, num_devices=NC)

    def din(name, shape, dt):
        return nc.dram_tensor(name, shape, dt, kind="ExternalInput")

    xt_in = din("xt", [128, BLK], f32)
    xo_in = din("xo", [16, NMAX], f32)
    ge1_in = din("ge1", [128, NCHUNK1 * ECH1 // 16], i16)
    gd1_in = din("gd1", [128, NCHUNK1 * NCH1 // 16], i16)
    ge2_in = din("ge2", [128, NCHUNK2 * ECH2 // 16], i16)
    gd2_in = din("gd2", [128, NCHUNK2 * NCH2 // 16], i16)
    gp_in = din("gp", [32, 4 * GPAD // 16], i16)
    w_ins = {}
    for nm, shape, dt in (
            ("w1e", [16, 16], f32), ("w1o", [16, 16], f32),
            ("b1e", [16, 1], f32), ("b1o", [16, 1], f32),
            ("w2e", [16, H], f32), ("w2o", [16, H], f32), ("b2", [H, 1], f32),
            ("gw1", [H, H], f32), ("gb1", [H, 1], f32),
            ("gw2", [H, H], f32), ("gb2", [H, 1], f32),
            ("gw3r", [H, H], f32), ("gb3c", [H, 1], f32),
            ("aw1", [H, H], f32), ("ab1", [H, 1], f32),
            ("aw2", [H, H], f32), ("ab2", [H, 1], f32),
            ("fw1", [H, H], f32), ("fb1", [H, 1], f32),
            ("fw2", [H, H], f32), ("fb2", [H, 1], f32),
            ("fw3r", [H, H], f32), ("fb3", [H, 1], f32),
            ("onesblk", [128, 16], f32), ("eye16", [16, 16], f32),
            ("eye16h", [16, 16], f16)):
        w_ins[nm] = din(nm, shape, dt)

    out_g = nc.dram_tensor("outg", [1, GW], f32, kind="ExternalOutput")

    h1i_own = nc.dram_tensor("h1i_own", [16, NMAX, 2], f16)
    h1i_all = nc.dram_tensor("h1i_all", [NC * 16, NMAX, 2], f16, addr_space="Shared")
    h2_dram = nc.dram_tensor("h2d", [H, NMAX], f32)

    with tile.TileContext(nc) as tc:
        with (
            tc.tile_pool(name="sp", bufs=1) as sp,
            tc.tile_pool(name="wp", bufs=2) as wp,
            tc.tile_pool(name="wq", bufs=1) as wq,
            tc.tile_pool(name="pp", bufs=2, space="PSUM") as pp,
        ):
            W = {}
            for nm in ("w1e", "w1o", "b1e", "b1o", "w2e", "w2o", "b2",
                       "onesblk", "eye16", "eye16h"):
                t_in = w_ins[nm]
                W[nm] = sp.tile(list(t_in.shape), t_in.dtype, name=f"w_{nm}")
                nc.sync.dma_start(W[nm][:], t_in.ap()[:])

            with tc.tile_pool(name="tbl", bufs=1) as tblp:
                # ---------------- Layer 1 ----------------
                table1 = tblp.tile([128, BLK], f32, tag="table")
                nc.sync.dma_start(table1[:], xt_in.ap()[:])

                with nc.named_scope("L1"):
                    for ch in range(NCHUNK1):
                        gidx = wp.tile([128, ECH1 // 16], i16, tag="gidx")
                        nc.sync.dma_start(
                            gidx[:],
                            ge1_in.ap()[:, ch * ECH1 // 16:(ch + 1) * ECH1 // 16])
                        didx = wp.tile([128, NCH1 // 16], i16, tag="didx")
                        nc.sync.dma_start(
                            didx[:],
                            gd1_in.ap()[:, ch * NCH1 // 16:(ch + 1) * NCH1 // 16])
                        xoc = wq.tile([16, NCH1], f32, tag="xoc")
                        nc.sync.dma_start(
                            xoc[:], xo_in.ap()[:, ch * NCH1:(ch + 1) * NCH1])

                        cs = wq.tile([128, 1 + ECH1], f32, tag="cs")
                        nc.vector.memset(cs[:, 0:1], 0.0)
                        nc.gpsimd.ap_gather(
                            cs[:, 1:], table1[:], gidx[:],
                            channels=128, num_elems=BLK, d=1, num_idxs=ECH1)
                        nc.vector._custom_dve(
                            CUMSUM, out=cs[:, 1:], in0=cs[:, 1:], s0=0.0)

                        G = wq.tile([128, 1 + NCH1], f32, tag="G")
                        nc.vector.memset(G[:, 0:1], 0.0)
                        nc.gpsimd.ap_gather(
                            G[:, 1:], cs[:], didx[:],
                            channels=128, num_elems=1 + ECH1, d=1, num_idxs=NCH1)
                        P = wq.tile([128, NCH1], f32, tag="P")
                        nc.vector.tensor_tensor(P[:], G[:, 1:], G[:, :-1], SUB)

                        for t0 in range(0, NCH1, TILE_N):
                            tn = min(TILE_N, NCH1 - t0)
                            sl = slice(t0, t0 + tn)
                            pa = pp.tile([16, tn], f32, tag="pa")
                            nc.tensor.matmul(pa[:], W["onesblk"][:], P[:, sl],
                                             start=True, stop=False)
                            nc.tensor.matmul(pa[:], W["eye16"][:], xoc[:, sl],
                                             start=False, stop=True)
                            sa = wp.tile([16, tn], f32, tag="sa")
                            nc.vector.tensor_copy(sa[:], pa[:])
                            phe = pp.tile([16, tn], f32, tag="ph")
                            nc.tensor.matmul(phe[:], W["w1e"][:], sa[:],
                                             start=True, stop=True)
                            pho = pp.tile([16, tn], f32, tag="pho")
                            nc.tensor.matmul(pho[:], W["w1o"][:], sa[:],
                                             start=True, stop=True)
                            he = wp.tile([16, tn, 2], f16, tag="he")
                            nc.scalar.activation(he[:, :, 0], phe[:], RELU,
                                                 bias=W["b1e"][:])
                            nc.scalar.activation(he[:, :, 1], pho[:], RELU,
                                                 bias=W["b1o"][:])
                            col = ch * NCH1 + t0
                            nc.sync.dma_start(
                                h1i_own.ap()[:, col:col + tn, :], he[:])

                # ---------------- exchange ----------------
                with nc.named_scope("AG"):
                    nc.gpsimd.collective_compute(
                        "AllGather", mybir.AluOpType.bypass,
                        replica_groups=[list(range(NC))],
                        ins=[h1i_own.ap()[:]],
                        outs=[h1i_all.ap()[:]],
                    )

                # ---------------- table2 ----------------
                table2 = tblp.tile([128, BLK, 2], f16, tag="table")
                with nc.named_scope("T2"):
                    for k in range(NC):
                        lo, hi = k * BLK, (k + 1) * BLK
                        pos = lo
                        while pos < hi:
                            c2 = next(i for i in range(NC)
                                      if bounds[i] <= pos < bounds[i + 1])
                            seg_end = min(hi, bounds[c2 + 1])
                            ln = seg_end - pos
                            local = pos - bounds[c2]
                            nc.sync.dma_start(
                                table2[16 * k:16 * (k + 1),
                                       pos - lo:pos - lo + ln, :],
                                h1i_all.ap()[16 * c2:16 * (c2 + 1),
                                             local:local + ln, :])
                            pos = seg_end

                # ---------------- Layer 2 ----------------
                with nc.named_scope("L2"):
                    for ch in range(NCHUNK2):
                        gidx = wp.tile([128, ECH2 // 16], i16, tag="gidx")
                        nc.sync.dma_start(
                            gidx[:],
                            ge2_in.ap()[:, ch * ECH2 // 16:(ch + 1) * ECH2 // 16])
                        didx = wp.tile([128, NCH2 // 16], i16, tag="didx")
                        nc.sync.dma_start(
                            didx[:],
                            gd2_in.ap()[:, ch * NCH2 // 16:(ch + 1) * NCH2 // 16])
                        h1c = wq.tile([16, NCH2, 2], f16, tag="xoc")
                        nc.sync.dma_start(
                            h1c[:], h1i_own.ap()[:, ch * NCH2:(ch + 1) * NCH2, :])

                        stage = wq.tile([128, ECH2, 2], f16, tag="stage")
                        nc.gpsimd.ap_gather(
                            stage[:], table2[:], gidx[:],
                            channels=128, num_elems=BLK, d=2, num_idxs=ECH2)
                        cs2 = wq.tile([128, 1 + ECH2, 2], f32, tag="cs")
                        nc.vector.memset(cs2[:, 0:1, :], 0.0)
                        nc.vector._custom_dve(
                            CUMSUM, out=cs2[:, 1:, 0], in0=stage[:, :, 0], s0=0.0)
                        nc.vector._custom_dve(
                            CUMSUM, out=cs2[:, 1:, 1], in0=stage[:, :, 1], s0=0.0)

                        G2 = wq.tile([128, 1 + NCH2, 2], f32, tag="G")
                        nc.vector.memset(G2[:, 0:1, :], 0.0)
                        nc.gpsimd.ap_gather(
                            G2[:, 1:, :], cs2[:], didx[:],
                            channels=128, num_elems=1 + ECH2, d=2, num_idxs=NCH2)
                        P2 = wq.tile([128, NCH2, 2], f32, tag="P")
                        nc.vector.tensor_tensor(P2[:], G2[:, 1:, :], G2[:, :-1, :],
                                                SUB)

                        for t0 in range(0, NCH2, TILE_N):
                            tn = min(TILE_N, NCH2 - t0)
                            sl = slice(t0, t0 + tn)
                            pe = pp.tile([16, tn], f32, tag="pa")
                            nc.tensor.matmul(pe[:], W["onesblk"][:], P2[:, sl, 0],
                                             start=True, stop=False)
                            nc.tensor.matmul(pe[:], W["eye16h"][:], h1c[:, sl, 0],
                                             start=False, stop=True)
                            po = pp.tile([16, tn], f32, tag="po")
                            nc.tensor.matmul(po[:], W["onesblk"][:], P2[:, sl, 1],
                                             start=True, stop=False)
                            nc.tensor.matmul(po[:], W["eye16h"][:], h1c[:, sl, 1],
                                             start=False, stop=True)
                            se = wp.tile([16, tn], f32, tag="sa")
                            so = wp.tile([16, tn], f32, tag="so")
                            nc.vector.tensor_copy(se[:], pe[:])
                            nc.vector.tensor_copy(so[:], po[:])
                            ph2 = pp.tile([H, tn], f32, tag="ph")
                            nc.tensor.matmul(ph2[:], W["w2e"][:], se[:],
                                             start=True, stop=False)
                            nc.tensor.matmul(ph2[:], W["w2o"][:], so[:],
                                             start=False, stop=True)
                            h2t = wp.tile([H, tn], f32, tag="he")
                            nc.scalar.activation(h2t[:], ph2[:], RELU,
                                                 bias=W["b2"][:])
                            col = ch * NCH2 + t0
                            nc.sync.dma_start(
                                h2_dram.ap()[:, col:col + tn], h2t[:])

            # ---------------- pooling (tbl pool closed) ----------------
            with tc.tile_pool(name="pool3", bufs=2) as p3:
                for nm in ("gw1", "gb1", "gw2", "gb2", "gw3r", "gb3c",
                           "aw1", "ab1", "aw2", "ab2",
                           "fw1", "fb1", "fw2", "fb2", "fw3r", "fb3"):
                    t_in = w_ins[nm]
                    W[nm] = p3.tile(list(t_in.shape), t_in.dtype, name=f"w_{nm}", bufs=1)
                    nc.sync.dma_start(W[nm][:], t_in.ap()[:])
                gpidx = p3.tile([32, 4 * GPAD // 16], i16, bufs=1)
                nc.sync.dma_start(gpidx[:], gp_in.ap()[:])

                G3w = p3.tile([H, 1 + GW], f32, bufs=1)
                G3e = p3.tile([H, 1 + GW], f32, bufs=1)
                nc.vector.memset(G3w[:, 0:1], 0.0)
                nc.vector.memset(G3e[:, 0:1], 0.0)
                carry_w = p3.tile([H, 1], f32, bufs=1)
                carry_e = p3.tile([H, 1], f32, bufs=1)
                nc.vector.memset(carry_w[:], 0.0)
                nc.vector.memset(carry_e[:], 0.0)

                with nc.named_scope("POOL"):
                    for ch in range(4):
                        csw = p3.tile([H, 1 + NCH3], f32, tag="csw", bufs=1)
                        cse = p3.tile([H, 1 + NCH3], f32, tag="cse", bufs=1)
                        # col 0 holds the carry (for empty-chunk pad idx 0)
                        nc.vector.tensor_copy(csw[:, 0:1], carry_w[:])
                        nc.vector.tensor_copy(cse[:, 0:1], carry_e[:])
                        for t0 in range(0, NCH3, TILE_N):
                            tn = min(TILE_N, NCH3 - t0)
                            col = ch * NCH3 + t0
                            h2c = p3.tile([H, tn], f32, tag="h2c")
                            nc.sync.dma_start(
                                h2c[:], h2_dram.ap()[:, col:col + tn])
                            pg = pp.tile([H, tn], f32, tag="ph")
                            nc.tensor.matmul(pg[:], W["gw1"][:], h2c[:],
                                             start=True, stop=True)
                            g1 = p3.tile([H, tn], f32, tag="g1")
                            nc.scalar.activation(g1[:], pg[:], RELU,
                                                 bias=W["gb1"][:])
                            pg2 = pp.tile([H, tn], f32, tag="ph")
                            nc.tensor.matmul(pg2[:], W["gw2"][:], g1[:],
                                             start=True, stop=True)
                            g2 = p3.tile([H, tn], f32, tag="g2")
                            nc.scalar.activation(g2[:], pg2[:], RELU,
                                                 bias=W["gb2"][:])
                            pg3 = pp.tile([H, tn], f32, tag="ph")
                            nc.tensor.matmul(pg3[:], W["gw3r"][:], g2[:],
                                             start=True, stop=True)
                            ee = p3.tile([H, tn], f32, tag="ee")
                            nc.scalar.activation(ee[:], pg3[:], EXP,
                                                 bias=W["gb3c"][:])
                            pt = pp.tile([H, tn], f32, tag="ph")
                            nc.tensor.matmul(pt[:], W["aw1"][:], h2c[:],
                                             start=True, stop=True)
                            t1 = p3.tile([H, tn], f32, tag="g1")
                            nc.scalar.activation(t1[:], pt[:], RELU,
                                                 bias=W["ab1"][:])
                            pt2 = pp.tile([H, tn], f32, tag="ph")
                            nc.tensor.matmul(pt2[:], W["aw2"][:], t1[:],
                                             start=True, stop=True)
                            t2 = p3.tile([H, tn], f32, tag="g2")
                            nc.scalar.activation(t2[:], pt2[:], RELU,
                                                 bias=W["ab2"][:])
                            wt = p3.tile([H, tn], f32, tag="wt")
                            nc.vector.tensor_tensor(wt[:], ee[:], t2[:], MUL)
                            s0w = carry_w[:] if t0 == 0 else csw[:, t0:t0 + 1]
                            s0e = carry_e[:] if t0 == 0 else cse[:, t0:t0 + 1]
                            nc.vector._custom_dve(
                                CUMSUM, out=csw[:, t0 + 1:t0 + 1 + tn],
                                in0=wt[:], s0=s0w)
                            nc.vector._custom_dve(
                                CUMSUM, out=cse[:, t0 + 1:t0 + 1 + tn],
                                in0=ee[:], s0=s0e)
                        gsl = slice(ch * GPAD // 16, (ch + 1) * GPAD // 16)
                        nc.gpsimd.ap_gather(
                            G3w[:, 1 + ch * GPAD:1 + (ch + 1) * GPAD],
                            csw[:], gpidx[:, gsl],
                            channels=32, num_elems=1 + NCH3, d=1, num_idxs=GPAD)
                        nc.gpsimd.ap_gather(
                            G3e[:, 1 + ch * GPAD:1 + (ch + 1) * GPAD],
                            cse[:], gpidx[:, gsl],
                            channels=32, num_elems=1 + NCH3, d=1, num_idxs=GPAD)
                        nc.vector.tensor_copy(carry_w[:], csw[:, NCH3:NCH3 + 1])
                        nc.vector.tensor_copy(carry_e[:], cse[:, NCH3:NCH3 + 1])

                    pooled = p3.tile([H, GW], f32, bufs=1)
                    denom = p3.tile([H, GW], f32, bufs=1)
                    nc.vector.tensor_tensor(pooled[:], G3w[:, 1:], G3w[:, :-1], SUB)
                    nc.vector.tensor_tensor(denom[:], G3e[:, 1:], G3e[:, :-1], SUB)
                    rec = p3.tile([H, GW], f32, bufs=1)
                    nc.vector.reciprocal(rec[:], denom[:])
                    atth = p3.tile([H, GW], f32, bufs=1)
                    nc.vector.tensor_tensor(atth[:], pooled[:], rec[:], MUL)
                    pf = pp.tile([H, GW], f32, tag="ph")
                    nc.tensor.matmul(pf[:], W["fw1"][:], atth[:],
                                     start=True, stop=True)
                    o1 = p3.tile([H, GW], f32, bufs=1)
                    nc.scalar.activation(o1[:], pf[:], RELU, bias=W["fb1"][:])
                    pf2 = pp.tile([H, GW], f32, tag="ph")
                    nc.tensor.matmul(pf2[:], W["fw2"][:], o1[:],
                                     start=True, stop=True)
                    o2 = p3.tile([H, GW], f32, bufs=1)
                    nc.scalar.activation(o2[:], pf2[:], RELU, bias=W["fb2"][:])
                    pf3 = pp.tile([H, GW], f32, tag="ph")
                    nc.tensor.matmul(pf3[:], W["fw3r"][:], o2[:],
                                     start=True, stop=True)
                    o3 = p3.tile([H, GW], f32, bufs=1)
                    nc.vector.tensor_scalar_add(o3[:], pf3[:], W["fb3"][:])
                    nc.sync.dma_start(out_g.ap()[:], o3[0:1, :])

    nc.compile()
    _split_multi_waits(nc, mybir)
    return nc


# ================================================================ entry
def kernel(x, w1, b1, w2, b2, gw1, gb1, gw2, gb2, gw3, gb3,
           aw1, ab1, aw2, ab2, fw1, fb1, fw2, fb2, fw3, fb3,
           edge_index, batch_vec, num_graphs):
    from concourse.bass_utils import run_bass_kernel_spmd

    x = np.asarray(x, np.float32)
    cores, bounds = _prep(edge_index, batch_vec)

    w1n = np.asarray(w1, np.float32)
    w1e_h = np.ascontiguousarray(w1n[:, 0::2])
    w1o_h = np.ascontiguousarray(w1n[:, 1::2])
    b1n = np.asarray(b1, np.float32)
    b1e_h = np.ascontiguousarray(b1n[0::2].reshape(16, 1))
    b1o_h = np.ascontiguousarray(b1n[1::2].reshape(16, 1))
    w2n = np.asarray(w2, np.float32)
    w2e = np.ascontiguousarray(w2n[0::2, :])
    w2o = np.ascontiguousarray(w2n[1::2, :])

    xt = np.zeros((128, BLK), np.float32)
    for k in range(NC):
        xt[16 * k:16 * (k + 1), :] = x[BLK * k:BLK * (k + 1), :].T

    ones_blk = np.zeros((128, 16), np.float32)
    for p in range(128):
        ones_blk[p, p % 16] = 1.0
    eye16 = np.eye(16, dtype=np.float32)

    gw3r = np.tile(np.asarray(gw3, np.float32).reshape(H, 1), (1, H))
    fw3r = np.tile(np.asarray(fw3, np.float32).reshape(H, 1), (1, H))
    gb3c = np.full((H, 1),
                   float(np.asarray(gb3).reshape(-1)[0]) - SOFTMAX_SHIFT, np.float32)
    fb3c = np.full((H, 1), float(np.asarray(fb3).reshape(-1)[0]), np.float32)

    def f16a(a):
        return np.ascontiguousarray(np.asarray(a, np.float32).astype(np.float16))

    def colb(a):
        return np.ascontiguousarray(np.asarray(a, np.float32).reshape(H, 1))

    common = dict(
        xt=xt, w1e=w1e_h, w1o=w1o_h, b1e=b1e_h, b1o=b1o_h,
        w2e=w2e, w2o=w2o, b2=colb(b2),
        gw1=np.asarray(gw1, np.float32), gb1=colb(gb1),
        gw2=np.asarray(gw2, np.float32), gb2=colb(gb2),
        gw3r=gw3r, gb3c=gb3c,
        aw1=np.asarray(aw1, np.float32), ab1=colb(ab1),
        aw2=np.asarray(aw2, np.float32), ab2=colb(ab2),
        fw1=np.asarray(fw1, np.float32), fb1=colb(fb1),
        fw2=np.asarray(fw2, np.float32), fb2=colb(fb2),
        fw3r=fw3r, fb3=fb3c,
        onesblk=ones_blk, eye16=eye16, eye16h=eye16.astype(np.float16),
    )

    in_maps = []
    for c, info in enumerate(cores):
        xo = np.zeros((16, NMAX), np.float32)
        xo[:, :info['size']] = x[info['n_lo']:info['n_lo'] + info['size'], :].T
        m = dict(common)
        m.update(xo=xo, ge1=info['ge1'], gd1=info['gd1'],
                 ge2=info['ge2'], gd2=info['gd2'], gp=info['gp'])
        in_maps.append(m)

    key = tuple(bounds)
    if _cache.get('key') != key:
        _cache['nc'] = _build_program(bounds)
        _cache['key'] = key
    nc = _cache['nc']

    res = run_bass_kernel_spmd(nc, in_maps, core_ids=list(range(NC)),
                               trace=bool(os.environ.get("KERNEL_TRACE")))
    _cache['last_results'] = res

    out = np.zeros((N_GRAPHS, 1), np.float32)
    for c, info in enumerate(cores):
        vals = np.asarray(res.results[c]["outg"]).reshape(-1)
        for s, g in enumerate(info['slot_map']):
            if g >= 0:
                out[info['g_lo'] + g, 0] = vals[s]
    return out



# revision 2
# speedup vs baseline: 3.8710x; 3.8710x over previous
"""2-layer GIN + attentional pooling on 8 Trainium2 NeuronCores (Bass/Tile).

Architecture (no usable DMA scatter/gather on this runtime):
  - Nodes split into 8 graph-aligned ownership ranges (one per core); each
    core processes edges whose dst it owns.
  - Per core, edges are bucketed by fixed 32768-node src blocks (one per
    GPSIMD core-group) and ordered by dst within uniform node-chunks.
  - x / h1 live in SBUF as per-block gather tables; GPSIMD ap_gather fetches
    per-edge source rows (feature-major).
  - Segment sums over dst: DVE prefix-scan over the dst-sorted edge stream,
    ap_gather of the cumsum at host-known segment-end positions (stream in
    quarter-interleaved order so each 4-index read request touches far-apart
    SBUF addresses), strided-shift subtraction un-permutes. Per-block group
    partials fold via a block-ones PE matmul.
  - L1/L2 loops are software-pipelined: chunk k+1's edge gather is issued
    before chunk k's segment-end gather so GPSIMD never waits on the DVE
    cumsum.
  - h1 exchanged across cores with AllGather.
  - Pooling: exact per-graph softmax via one-hot selector matmuls: per-node
    exp(gate-20) and exp*transform accumulate into per-graph PSUM slots
    (numerator 32 rows + denominator row); no cross-graph running sums.
"""
import os
import sys

os.environ.setdefault("NEURON_RT_RESET_CORES", "1")
sys.path.insert(0, '/opt/trn_rl_repo')

import numpy as np


# -- NTFF profiling hook shim (optional; enables trace=True under axon) ----
def _install_ntff_shim():
    import types
    try:
        import antenv
        if 'antenv.axon_hooks' in sys.modules:
            return
        hooks = types.ModuleType('antenv.axon_hooks')
        _state = {'hook': None}
        hooks.set_axon_ntff_profile_hook = lambda h: _state.__setitem__('hook', h)
        hooks.get_axon_ntff_profile_hook = lambda: _state['hook']
        sys.modules['antenv.axon_hooks'] = hooks
        antenv.axon_hooks = hooks
        from trn_agent_boot.trn_boot import _ntff_profile_via_ctypes
        h = _ntff_profile_via_ctypes('/opt/axon/libaxon_pjrt.so')
        if h is not None:
            hooks.set_axon_ntff_profile_hook(h)
    except Exception:
        pass


_install_ntff_shim()

N_NODES = 262144
N_GRAPHS = 1024
C_IN = 16
H = 32
NC = 8
BLK = 32768
NCHUNK1, NCH1, ECH1 = 16, 2112, 4608
NCHUNK2, NCH2, ECH2 = 32, 1056, 2432
NMAX = NCHUNK1 * NCH1            # 33792
Q1 = NCH1 // 4                   # 528
Q2 = NCH2 // 4                   # 264
GW = 256                         # graph slots per core
NPC = NMAX // 128                # 264 pool node-chunks
SOFTMAX_SHIFT = 20.0
MAX_WAITS = 1
TILE_N = 512

_cache = {}


def _split_multi_waits(nc, mybir, max_waits=MAX_WAITS):
    n_split = 0
    for fn in nc.m.functions:
        for bb in fn.blocks:
            out = []
            for ins in bb.instructions:
                si = ins.sync_info
                if si is not None and si.on_wait and len(si.on_wait) > max_waits:
                    waits = list(si.on_wait)
                    extra = waits[:-max_waits]
                    keep = waits[-max_waits:]
                    for i in range(0, len(extra), max_waits):
                        group = extra[i:i + max_waits]
                        nop = mybir.InstNoOp(
                            name=f"waitsplit_{nc.next_id()}",
                            sync_info=mybir.SyncInfo(on_wait=group, on_update=[]),
                            bass_nofuse=True,
                            engine=ins.engine,
                        )
                        out.append(nop)
                        n_split += 1
                    si.on_wait = keep
                out.append(ins)
            bb.instructions = out
    return n_split


def _wrap_idx(vals, group, arr, col0=0):
    """Wrapped ap_gather index layout: value i -> arr[16g + i%16, col0 + i//16]."""
    n = len(vals)
    assert n % 16 == 0
    v = np.asarray(vals, dtype=np.int16).reshape(n // 16, 16).T
    arr[16 * group:16 * group + 16, col0:col0 + n // 16] = v


def _register_cumsum():
    from concourse import dve_ops
    from concourse.dve_spec import Spec, Src0, C0, AluOp, lower
    import concourse.dve_spec as ds
    from concourse.dve_uop import DveOpSpec
    for op in dve_ops.OPS:
        if op.name == "CUMSUM_ANT":
            return op
    spec = Spec(
        body=ds.scan(AluOp.ADD, Src0, init=C0),
        reference=lambda in0, s0: np.cumsum(in0.astype(np.float32), axis=-1) + s0,
    )
    shas = {}
    for ver in ("v3", "v4"):
        uops = lower(spec, ver=ver)
        shas[ver] = DveOpSpec(name="CUMSUM_ANT", opcode=1, uops=uops,
                              rd1_en=False).sha(ver)
    op = dve_ops.DveOp("CUMSUM_ANT", spec, subdim=False, uops_sha=shas)
    dve_ops.OPS.append(op)
    dve_ops.CUSTOM_DVE_SPECS["CUMSUM_ANT"] = spec
    dve_ops._SUB_OPCODE_FOR_NAME["CUMSUM_ANT"] = \
        max(dve_ops._SUB_OPCODE_FOR_NAME.values()) + 1
    return op


def _sigma(nch):
    """Stream position i gathers the end of node sigma(i); i = 4j+k ->
    node k*(nch//4) + j (so one 4-index read request spans the table)."""
    i = np.arange(nch)
    return (i % 4) * (nch // 4) + i // 4


# ================================================================ host prep
def _prep(edge_index, batch_vec):
    src = np.asarray(edge_index[0], dtype=np.int64)
    dst = np.asarray(edge_index[1], dtype=np.int64)
    bv = np.asarray(batch_vec, dtype=np.int64)

    gstart = np.searchsorted(bv, np.arange(N_GRAPHS))
    bounds = [0]
    for c in range(1, NC):
        target = c * (N_NODES // NC)
        gi = np.searchsorted(gstart, target)
        cand = []
        if gi < N_GRAPHS:
            cand.append(int(gstart[gi]))
        if gi > 0:
            cand.append(int(gstart[gi - 1]))
        bounds.append(min(cand, key=lambda v: abs(v - target)))
    bounds.append(N_NODES)
    n_lo = np.array(bounds[:-1])
    n_hi = np.array(bounds[1:])
    sizes = n_hi - n_lo
    assert sizes.max() <= NMAX, sizes
    g_lo = np.searchsorted(gstart, n_lo)
    g_hi = np.searchsorted(gstart, n_hi)

    owner = np.searchsorted(n_hi, dst, side='right')

    sig1 = _sigma(NCH1)
    sig2 = _sigma(NCH2)

    cores = []
    for c in range(NC):
        m = owner == c
        csrc = src[m]
        cdst_local = dst[m] - n_lo[c]
        size_c = int(sizes[c])

        ge1 = np.zeros((128, NCHUNK1 * ECH1 // 16), np.int16)
        gd1 = np.zeros((128, NCHUNK1 * NCH1 // 16), np.int16)
        ge2 = np.zeros((128, NCHUNK2 * ECH2 // 16), np.int16)
        gd2 = np.zeros((128, NCHUNK2 * NCH2 // 16), np.int16)

        blk_of = csrc >> 15
        src_local_all = (csrc & (BLK - 1))

        for k in range(NC):
            bm = blk_of == k
            bsrc = src_local_all[bm]
            bdst = cdst_local[bm]
            order = np.argsort(bdst, kind='stable')
            bsrc = bsrc[order].astype(np.int16)
            bdst = bdst[order]
            cnt = np.bincount(bdst, minlength=NMAX)
            cum = np.concatenate([[0], np.cumsum(cnt)])

            for (nch, ech, nchunks, ge, gd, sig) in (
                    (NCH1, ECH1, NCHUNK1, ge1, gd1, sig1),
                    (NCH2, ECH2, NCHUNK2, ge2, gd2, sig2)):
                for ch in range(nchunks):
                    a, b = ch * nch, (ch + 1) * nch
                    e0, e1 = cum[a], cum[b]
                    ne = int(e1 - e0)
                    assert ne <= ech, (c, k, ch, ne, ech)
                    ev = np.empty(ech, np.int16)
                    ev[:ne] = bsrc[e0:e1]
                    npad = ech - ne
                    if npad:
                        # spread pad indices: same-address read pairs are slow
                        ev[ne:] = ((np.arange(npad) * 7919 + 131) % BLK) \
                            .astype(np.int16)
                    _wrap_idx(ev, k, ge, col0=ch * ech // 16)
                    ends = (cum[a + 1:b + 1] - e0).astype(np.int16)
                    _wrap_idx(ends[sig], k, gd, col0=ch * nch // 16)

        # ---- pooling: per-node graph slots + selector windows ----
        g_cnt = int(g_hi[c] - g_lo[c])
        assert g_cnt <= GW, g_cnt
        slots = (bv[n_lo[c]:n_hi[c]] - g_lo[c]).astype(np.int64)
        slots_pad = np.full(NMAX, -1, np.int64)
        slots_pad[:size_c] = slots
        gmin = np.zeros(NPC, np.int64)
        sel = np.zeros((128, NPC * 8), np.float32)
        for ch in range(NPC):
            sl = slots_pad[ch * 128:(ch + 1) * 128]
            valid = sl >= 0
            if valid.any():
                lo = int(sl[valid].min())
                hi = int(sl[valid].max())
                assert hi - lo < 8, (c, ch, lo, hi)
                lo = min(lo, GW - 8)
                gmin[ch] = lo
                for p in range(128):
                    if sl[p] >= 0:
                        sel[p, ch * 8 + (sl[p] - lo)] = 1.0
            else:
                gmin[ch] = 0

        cores.append(dict(
            n_lo=int(n_lo[c]), size=size_c, g_lo=int(g_lo[c]), g_cnt=g_cnt,
            ge1=ge1, gd1=gd1, ge2=ge2, gd2=gd2,
            gmin=[int(v) for v in gmin], sel=sel,
        ))
    return cores, [int(b) for b in bounds]


# ================================================================ device
def _build_program(bounds, gmins):
    from concourse import bacc, tile
    from concourse.bass import mybir

    CUMSUM = _register_cumsum()

    f32 = mybir.dt.float32
    f16 = mybir.dt.float16
    bf16 = mybir.dt.bfloat16
    i16 = mybir.dt.int16
    RELU = mybir.ActivationFunctionType.Relu
    EXP = mybir.ActivationFunctionType.Exp
    SUB = mybir.AluOpType.subtract
    MUL = mybir.AluOpType.mult

    nc = bacc.Bacc("TRN2", target_bir_lowering=False, debug=False,
                   num_devices=NC)

    def din(name, shape, dt):
        return nc.dram_tensor(name, shape, dt, kind="ExternalInput")

    xt_in = din("xt", [128, BLK], f32)
    xo_in = din("xo", [16, NMAX], f32)
    ge1_in = din("ge1", [128, NCHUNK1 * ECH1 // 16], i16)
    gd1_in = din("gd1", [128, NCHUNK1 * NCH1 // 16], i16)
    ge2_in = din("ge2", [128, NCHUNK2 * ECH2 // 16], i16)
    gd2_in = din("gd2", [128, NCHUNK2 * NCH2 // 16], i16)
    sel_in = din("sel", [128, NPC * 8], bf16)
    w_ins = {}
    for nm, shape, dt in (
            ("w1e", [16, 16], f32), ("w1o", [16, 16], f32),
            ("b1e", [16, 1], f32), ("b1o", [16, 1], f32),
            ("w2e", [16, H], f32), ("w2o", [16, H], f32), ("b2", [H, 1], f32),
            ("gw1", [H, H], f32), ("gb1", [H, 1], f32),
            ("gw2", [H, H], f32), ("gb2", [H, 1], f32),
            ("gw3b", [H + 1, 1], f32),
            ("aw1", [H, H], f32), ("ab1", [H, 1], f32),
            ("aw2b", [H + 1, H], f32),
            ("fw1", [H, H], f32), ("fb1", [H, 1], f32),
            ("fw2", [H, H], f32), ("fb2", [H, 1], f32),
            ("fw3r", [H, H], f32), ("fb3", [H, 1], f32),
            ("bsh", [128, 1], f32),
            ("ones32", [1, H], f32), ("z33", [1, 33], bf16),
            ("z256", [1, GW], bf16),
            ("onesblk", [128, 16], f32), ("eye16", [16, 16], f32),
            ("eye16h", [16, 16], f16)):
        w_ins[nm] = din(nm, shape, dt)

    out_g = nc.dram_tensor("outg", [1, GW], f32, kind="ExternalOutput")

    h1i_own = nc.dram_tensor("h1i_own", [16, NMAX, 2], f16)
    h1i_all = nc.dram_tensor("h1i_all", [NC * 16, NMAX, 2], f16,
                             addr_space="Shared")
    h2_dram = nc.dram_tensor("h2d", [H, NMAX], f32)

    with tile.TileContext(nc) as tc:
        with (
            tc.tile_pool(name="wp", bufs=1) as wp,
            tc.tile_pool(name="pp", bufs=2, space="PSUM") as pp,
        ):
            W = {}
            for nm, t_in in w_ins.items():
                W[nm] = wp.tile(list(t_in.shape), t_in.dtype, name=f"w_{nm}")
                nc.sync.dma_start(W[nm][:], t_in.ap()[:])

            with tc.tile_pool(name="tbl", bufs=1) as tblp, \
                 tc.tile_pool(name="wk", bufs=1) as wk:
                # ---------------- Layer 1 ----------------
                table1 = tblp.tile([128, BLK], f32, tag="table")
                nc.sync.dma_start(table1[:], xt_in.ap()[:])

                gidx1 = [wk.tile([128, ECH1 // 16], i16, name=f"gidx1_{b}")
                         for b in range(2)]
                gdx1 = [wk.tile([128, NCH1 // 16], i16, name=f"gdx1_{b}")
                        for b in range(2)]
                cs1 = [wk.tile([128, 1 + ECH1], f32, name=f"cs1_{b}")
                       for b in range(2)]
                G1 = wk.tile([128, Q1, 4], f32, name="G1")
                P1 = wk.tile([128, NCH1], f32, name="P1")
                xoc = [wk.tile([16, TILE_N], f32, name=f"xoc_{b}")
                       for b in range(2)]
                sa1 = [wk.tile([16, TILE_N], f32, name=f"sa1_{b}")
                       for b in range(2)]
                he1 = [wk.tile([16, TILE_N, 2], f16, name=f"he1_{b}")
                       for b in range(2)]
                for b in range(2):
                    nc.vector.memset(cs1[b][:, 0:1], 0.0)

                def l1_dma(k):
                    b = k % 2
                    nc.sync.dma_start(
                        gidx1[b][:],
                        ge1_in.ap()[:, k * ECH1 // 16:(k + 1) * ECH1 // 16])
                    nc.sync.dma_start(
                        gdx1[b][:],
                        gd1_in.ap()[:, k * NCH1 // 16:(k + 1) * NCH1 // 16])

                with nc.named_scope("L1"):
                    l1_dma(0)
                    for k in range(NCHUNK1 + 1):
                        if k < NCHUNK1:
                            b = k % 2
                            if k + 1 < NCHUNK1:
                                l1_dma(k + 1)
                            nc.gpsimd.ap_gather(
                                cs1[b][:, 1:], table1[:], gidx1[b][:],
                                channels=128, num_elems=BLK, d=1,
                                num_idxs=ECH1)
                        if k >= 1:
                            j = k - 1
                            bj = j % 2
                            nc.vector._custom_dve(
                                CUMSUM, out=cs1[bj][:, 1:], in0=cs1[bj][:, 1:],
                                s0=0.0)
                            nc.gpsimd.ap_gather(
                                G1[:], cs1[bj][:], gdx1[bj][:],
                                channels=128, num_elems=1 + ECH1, d=1,
                                num_idxs=NCH1)
                            # un-permute sigma + segment-difference
                            for a in range(4):
                                nc.vector.tensor_tensor(
                                    P1[:, a * Q1 + 1:(a + 1) * Q1],
                                    G1[:, 1:Q1, a], G1[:, 0:Q1 - 1, a], SUB)
                            nc.vector.tensor_tensor(
                                P1[:, Q1:3 * Q1 + 1:Q1],
                                G1[:, 0, 1:4], G1[:, Q1 - 1, 0:3], SUB)
                            nc.vector.tensor_copy(P1[:, 0:1], G1[:, 0, 0:1])

                            for t0 in range(0, NCH1, TILE_N):
                                tn = min(TILE_N, NCH1 - t0)
                                tb = (t0 // TILE_N) % 2
                                col = j * NCH1 + t0
                                nc.sync.dma_start(
                                    xoc[tb][:, 0:tn],
                                    xo_in.ap()[:, col:col + tn])
                                pa = pp.tile([16, TILE_N], f32, tag="psA")
                                nc.tensor.matmul(pa[:, 0:tn], W["onesblk"][:],
                                                 P1[:, t0:t0 + tn],
                                                 start=True, stop=False)
                                nc.tensor.matmul(pa[:, 0:tn], W["eye16"][:],
                                                 xoc[tb][:, 0:tn],
                                                 start=False, stop=True)
                                nc.vector.tensor_copy(sa1[tb][:, 0:tn],
                                                      pa[:, 0:tn])
                                phe = pp.tile([16, TILE_N], f32, tag="psB")
                                nc.tensor.matmul(phe[:, 0:tn], W["w1e"][:],
                                                 sa1[tb][:, 0:tn],
                                                 start=True, stop=True)
                                pho = pp.tile([16, TILE_N], f32, tag="psC")
                                nc.tensor.matmul(pho[:, 0:tn], W["w1o"][:],
                                                 sa1[tb][:, 0:tn],
                                                 start=True, stop=True)
                                nc.scalar.activation(he1[tb][:, 0:tn, 0],
                                                     phe[:, 0:tn], RELU,
                                                     bias=W["b1e"][:])
                                nc.scalar.activation(he1[tb][:, 0:tn, 1],
                                                     pho[:, 0:tn], RELU,
                                                     bias=W["b1o"][:])
                                nc.sync.dma_start(
                                    h1i_own.ap()[:, col:col + tn, :],
                                    he1[tb][:, 0:tn, :])

                # ---------------- exchange ----------------
                with nc.named_scope("AG"):
                    nc.gpsimd.collective_compute(
                        "AllGather", mybir.AluOpType.bypass,
                        replica_groups=[list(range(NC))],
                        ins=[h1i_own.ap()[:]],
                        outs=[h1i_all.ap()[:]],
                    )

                # ---------------- table2 ----------------
                table2 = tblp.tile([128, BLK, 2], f16, tag="table")
                with nc.named_scope("T2"):
                    for k in range(NC):
                        lo, hi = k * BLK, (k + 1) * BLK
                        pos = lo
                        while pos < hi:
                            c2 = next(i for i in range(NC)
                                      if bounds[i] <= pos < bounds[i + 1])
                            seg_end = min(hi, bounds[c2 + 1])
                            ln = seg_end - pos
                            local = pos - bounds[c2]
                            nc.sync.dma_start(
                                table2[16 * k:16 * (k + 1),
                                       pos - lo:pos - lo + ln, :],
                                h1i_all.ap()[16 * c2:16 * (c2 + 1),
                                             local:local + ln, :])
                            pos = seg_end

                # ---------------- Layer 2 ----------------
                gidx2 = [wk.tile([128, ECH2 // 16], i16, name=f"gidx2_{b}")
                         for b in range(2)]
                gdx2 = [wk.tile([128, NCH2 // 16], i16, name=f"gdx2_{b}")
                        for b in range(2)]
                stage = [wk.tile([128, ECH2, 2], f16, name=f"stage_{b}")
                         for b in range(2)]
                cs2 = wk.tile([128, 1 + ECH2, 2], f32, name="cs2")
                G2 = wk.tile([128, Q2, 4, 2], f32, name="G2")
                P2 = wk.tile([128, NCH2, 2], f32, name="P2")
                h1c = [wk.tile([16, TILE_N, 2], f16, name=f"h1c_{b}")
                       for b in range(2)]
                se2 = [wk.tile([16, TILE_N], f32, name=f"se2_{b}")
                       for b in range(2)]
                so2 = [wk.tile([16, TILE_N], f32, name=f"so2_{b}")
                       for b in range(2)]
                h2t = [wk.tile([H, TILE_N], f32, name=f"h2t_{b}")
                       for b in range(2)]
                nc.vector.memset(cs2[:, 0:1, :], 0.0)

                def l2_dma(k):
                    b = k % 2
                    nc.sync.dma_start(
                        gidx2[b][:],
                        ge2_in.ap()[:, k * ECH2 // 16:(k + 1) * ECH2 // 16])
                    nc.sync.dma_start(
                        gdx2[b][:],
                        gd2_in.ap()[:, k * NCH2 // 16:(k + 1) * NCH2 // 16])

                with nc.named_scope("L2"):
                    l2_dma(0)
                    for k in range(NCHUNK2 + 1):
                        if k < NCHUNK2:
                            b = k % 2
                            if k + 1 < NCHUNK2:
                                l2_dma(k + 1)
                            nc.gpsimd.ap_gather(
                                stage[b][:], table2[:], gidx2[b][:],
                                channels=128, num_elems=BLK, d=2,
                                num_idxs=ECH2)
                        if k >= 1:
                            j = k - 1
                            bj = j % 2
                            nc.vector._custom_dve(
                                CUMSUM, out=cs2[:, 1:, 0],
                                in0=stage[bj][:, :, 0], s0=0.0)
                            nc.vector._custom_dve(
                                CUMSUM, out=cs2[:, 1:, 1],
                                in0=stage[bj][:, :, 1], s0=0.0)
                            nc.gpsimd.ap_gather(
                                G2[:], cs2[:], gdx2[bj][:],
                                channels=128, num_elems=1 + ECH2, d=2,
                                num_idxs=NCH2)
                            for a in range(4):
                                nc.vector.tensor_tensor(
                                    P2[:, a * Q2 + 1:(a + 1) * Q2, :],
                                    G2[:, 1:Q2, a, :], G2[:, 0:Q2 - 1, a, :],
                                    SUB)
                            nc.vector.tensor_tensor(
                                P2[:, Q2:3 * Q2 + 1:Q2, :],
                                G2[:, 0, 1:4, :], G2[:, Q2 - 1, 0:3, :], SUB)
                            nc.vector.tensor_copy(P2[:, 0:1, :],
                                                  G2[:, 0, 0:1, :])

                            for t0 in range(0, NCH2, TILE_N):
                                tn = min(TILE_N, NCH2 - t0)
                                tb = (t0 // TILE_N) % 2
                                col = j * NCH2 + t0
                                nc.sync.dma_start(
                                    h1c[tb][:, 0:tn, :],
                                    h1i_own.ap()[:, col:col + tn, :])
                                pe = pp.tile([16, TILE_N], f32, tag="psA")
                                nc.tensor.matmul(pe[:, 0:tn], W["onesblk"][:],
                                                 P2[:, t0:t0 + tn, 0],
                                                 start=True, stop=False)
                                nc.tensor.matmul(pe[:, 0:tn], W["eye16h"][:],
                                                 h1c[tb][:, 0:tn, 0],
                                                 start=False, stop=True)
                                po = pp.tile([16, TILE_N], f32, tag="psB")
                                nc.tensor.matmul(po[:, 0:tn], W["onesblk"][:],
                                                 P2[:, t0:t0 + tn, 1],
                                                 start=True, stop=False)
                                nc.tensor.matmul(po[:, 0:tn], W["eye16h"][:],
                                                 h1c[tb][:, 0:tn, 1],
                                                 start=False, stop=True)
                                nc.vector.tensor_copy(se2[tb][:, 0:tn],
                                                      pe[:, 0:tn])
                                nc.vector.tensor_copy(so2[tb][:, 0:tn],
                                                      po[:, 0:tn])
                                ph2 = pp.tile([H, TILE_N], f32, tag="psC")
                                nc.tensor.matmul(ph2[:, 0:tn], W["w2e"][:],
                                                 se2[tb][:, 0:tn],
                                                 start=True, stop=False)
                                nc.tensor.matmul(ph2[:, 0:tn], W["w2o"][:],
                                                 so2[tb][:, 0:tn],
                                                 start=False, stop=True)
                                nc.scalar.activation(h2t[tb][:, 0:tn],
                                                     ph2[:, 0:tn], RELU,
                                                     bias=W["b2"][:])
                                nc.sync.dma_start(
                                    h2_dram.ap()[:, col:col + tn],
                                    h2t[tb][:, 0:tn])

            # ---------------- pooling ----------------
            with tc.tile_pool(name="p3", bufs=1) as p3, \
                 tc.tile_pool(name="pq", bufs=1, space="PSUM") as pq:
                sel_t = p3.tile([128, NPC * 8], bf16, name="sel")
                nc.sync.dma_start(sel_t[:], sel_in.ap()[:])

                pool_ps = pq.tile([33, GW], f32, name="pool_ps")
                nc.tensor.matmul(pool_ps[:], W["z33"][:], W["z256"][:],
                                 start=True, stop=False)

                h2c = [p3.tile([H, TILE_N], f32, name=f"h2c_{b}")
                       for b in range(3)]
                g1t = [p3.tile([H, TILE_N], f32, name=f"g1t_{b}")
                       for b in range(2)]
                g2e = [p3.tile([H + 1, TILE_N], f32, name=f"g2e_{b}")
                       for b in range(2)]
                t1e = [p3.tile([H + 1, TILE_N], f32, name=f"t1e_{b}")
                       for b in range(2)]
                ee = [p3.tile([128, 1], f32, name=f"ee_{b}")
                      for b in range(2)]
                t2r = [p3.tile([128, H], f32, name=f"t2r_{b}")
                       for b in range(2)]
                stk = [p3.tile([128, 33], bf16, name=f"stk_{b}")
                       for b in range(2)]
                for b in range(2):
                    nc.vector.memset(g2e[b][H:H + 1, :], 1.0)
                    nc.vector.memset(t1e[b][H:H + 1, :], 1.0)

                NT = NMAX // TILE_N
                with nc.named_scope("POOL"):
                    for t in range(NT):
                        b = t % 3
                        b2_ = t % 2
                        nc.sync.dma_start(
                            h2c[b][:],
                            h2_dram.ap()[:, t * TILE_N:(t + 1) * TILE_N])
                        pg = pp.tile([H, TILE_N], f32, tag="psA")
                        nc.tensor.matmul(pg[:], W["gw1"][:], h2c[b][:],
                                         start=True, stop=True)
                        nc.scalar.activation(g1t[b2_][:], pg[:], RELU,
                                             bias=W["gb1"][:])
                        pg2 = pp.tile([H, TILE_N], f32, tag="psB")
                        nc.tensor.matmul(pg2[:], W["gw2"][:], g1t[b2_][:],
                                         start=True, stop=True)
                        nc.scalar.activation(g2e[b2_][0:H, :], pg2[:], RELU,
                                             bias=W["gb2"][:])
                        pt = pp.tile([H, TILE_N], f32, tag="psC")
                        nc.tensor.matmul(pt[:], W["aw1"][:], h2c[b][:],
                                         start=True, stop=True)
                        nc.scalar.activation(t1e[b2_][0:H, :], pt[:], RELU,
                                             bias=W["ab1"][:])
                        for cc in range(4):
                            ci = t * 4 + cc
                            cb = cc % 2
                            sl = slice(cc * 128, (cc + 1) * 128)
                            pg3 = pp.tile([128, 1], f32, tag="psD")
                            nc.tensor.matmul(pg3[:], g2e[b2_][:, sl],
                                             W["gw3b"][:],
                                             start=True, stop=True)
                            nc.scalar.activation(ee[cb][:], pg3[:], EXP,
                                                 bias=W["bsh"][:])
                            pt2 = pp.tile([128, H], f32, tag="psE")
                            nc.tensor.matmul(pt2[:], t1e[b2_][:, sl],
                                             W["aw2b"][:],
                                             start=True, stop=True)
                            nc.scalar.activation(t2r[cb][:], pt2[:], RELU)
                            nc.vector.tensor_scalar_mul(
                                stk[cb][:, 0:H], t2r[cb][:], ee[cb][:])
                            nc.vector.tensor_copy(stk[cb][:, H:H + 1],
                                                  ee[cb][:])
                            w0 = gmins[ci]
                            nc.tensor.matmul(
                                pool_ps[:, w0:w0 + 8], stk[cb][:],
                                sel_t[:, ci * 8:(ci + 1) * 8],
                                start=False, stop=False)
                    nc.tensor.matmul(pool_ps[:], W["z33"][:], W["z256"][:],
                                     start=False, stop=True)

                    pool_sb = p3.tile([33, GW], f32, name="pool_sb")
                    nc.vector.tensor_copy(pool_sb[:], pool_ps[:])
                    pdn = pq.tile([H, GW], f32, name="pdn")
                    nc.tensor.matmul(pdn[:], W["ones32"][:],
                                     pool_sb[H:H + 1, :],
                                     start=True, stop=True)
                    rec = p3.tile([H, GW], f32, name="rec")
                    nc.vector.reciprocal(rec[:], pdn[:])
                    atth = p3.tile([H, GW], f32, name="atth")
                    nc.vector.tensor_tensor(atth[:], pool_sb[0:H, :], rec[:],
                                            MUL)
                    pf = pp.tile([H, GW], f32, tag="psA")
                    nc.tensor.matmul(pf[:], W["fw1"][:], atth[:],
                                     start=True, stop=True)
                    o1 = p3.tile([H, GW], f32, name="o1f")
                    nc.scalar.activation(o1[:], pf[:], RELU, bias=W["fb1"][:])
                    pf2 = pp.tile([H, GW], f32, tag="psB")
                    nc.tensor.matmul(pf2[:], W["fw2"][:], o1[:],
                                     start=True, stop=True)
                    o2 = p3.tile([H, GW], f32, name="o2f")
                    nc.scalar.activation(o2[:], pf2[:], RELU, bias=W["fb2"][:])
                    pf3 = pp.tile([H, GW], f32, tag="psC")
                    nc.tensor.matmul(pf3[:], W["fw3r"][:], o2[:],
                                     start=True, stop=True)
                    o3 = p3.tile([1, GW], f32, name="o3f")
                    nc.vector.tensor_scalar_add(o3[:], pf3[0:1, :],
                                                W["fb3"][0:1, :])
                    nc.sync.dma_start(out_g.ap()[:], o3[:])

    nc.compile()
    _split_multi_waits(nc, mybir)
    return nc


# ================================================================ entry
def kernel(x, w1, b1, w2, b2, gw1, gb1, gw2, gb2, gw3, gb3,
           aw1, ab1, aw2, ab2, fw1, fb1, fw2, fb2, fw3, fb3,
           edge_index, batch_vec, num_graphs):
    from concourse.bass_utils import run_bass_kernel_spmd

    x = np.asarray(x, np.float32)
    cores, bounds = _prep(edge_index, batch_vec)

    w1n = np.asarray(w1, np.float32)
    w1e_h = np.ascontiguousarray(w1n[:, 0::2])
    w1o_h = np.ascontiguousarray(w1n[:, 1::2])
    b1n = np.asarray(b1, np.float32)
    b1e_h = np.ascontiguousarray(b1n[0::2].reshape(16, 1))
    b1o_h = np.ascontiguousarray(b1n[1::2].reshape(16, 1))
    w2n = np.asarray(w2, np.float32)
    w2e = np.ascontiguousarray(w2n[0::2, :])
    w2o = np.ascontiguousarray(w2n[1::2, :])

    xt = np.zeros((128, BLK), np.float32)
    for k in range(NC):
        xt[16 * k:16 * (k + 1), :] = x[BLK * k:BLK * (k + 1), :].T

    ones_blk = np.zeros((128, 16), np.float32)
    for p in range(128):
        ones_blk[p, p % 16] = 1.0
    eye16 = np.eye(16, dtype=np.float32)

    gw3b = np.zeros((H + 1, 1), np.float32)
    gw3b[:H, 0] = np.asarray(gw3, np.float32).reshape(-1)
    gw3b[H, 0] = float(np.asarray(gb3).reshape(-1)[0])
    aw2b = np.zeros((H + 1, H), np.float32)
    aw2b[:H, :] = np.asarray(aw2, np.float32)
    aw2b[H, :] = np.asarray(ab2, np.float32).reshape(-1)
    fw3r = np.tile(np.asarray(fw3, np.float32).reshape(H, 1), (1, H))
    fb3c = np.full((H, 1), float(np.asarray(fb3).reshape(-1)[0]), np.float32)
    bsh = np.full((128, 1), -SOFTMAX_SHIFT, np.float32)

    def colb(a):
        return np.ascontiguousarray(np.asarray(a, np.float32).reshape(H, 1))

    common = dict(
        xt=xt, w1e=w1e_h, w1o=w1o_h, b1e=b1e_h, b1o=b1o_h,
        w2e=w2e, w2o=w2o, b2=colb(b2),
        gw1=np.asarray(gw1, np.float32), gb1=colb(gb1),
        gw2=np.asarray(gw2, np.float32), gb2=colb(gb2),
        gw3b=gw3b, bsh=bsh,
        aw1=np.asarray(aw1, np.float32), ab1=colb(ab1),
        aw2b=aw2b,
        fw1=np.asarray(fw1, np.float32), fb1=colb(fb1),
        fw2=np.asarray(fw2, np.float32), fb2=colb(fb2),
        fw3r=fw3r, fb3=fb3c,
        ones32=np.ones((1, H), np.float32),
        z33=np.zeros((1, 33), np.float16).view(np.uint16).astype(np.float32)
            .astype(np.float16) * 0,
        z256=np.zeros((1, GW), np.float16),
        onesblk=ones_blk, eye16=eye16, eye16h=eye16.astype(np.float16),
    )
    # bf16 host arrays: represent as float32 then cast via ml_dtypes if
    # available, else pass float32 and let the runner convert.
    try:
        import ml_dtypes
        common["z33"] = np.zeros((1, 33), ml_dtypes.bfloat16)
        common["z256"] = np.zeros((1, GW), ml_dtypes.bfloat16)

        def to_bf16(a):
            return np.asarray(a, np.float32).astype(ml_dtypes.bfloat16)
    except ImportError:
        def to_bf16(a):
            return np.asarray(a, np.float32)

    in_maps = []
    gmins = None
    for c, info in enumerate(cores):
        xo = np.zeros((16, NMAX), np.float32)
        xo[:, :info['size']] = x[info['n_lo']:info['n_lo'] + info['size'], :].T
        m = dict(common)
        m.update(xo=xo, ge1=info['ge1'], gd1=info['gd1'],
                 ge2=info['ge2'], gd2=info['gd2'],
                 sel=to_bf16(info['sel']))
        in_maps.append(m)
        if gmins is None:
            gmins = info['gmin']
        else:
            assert gmins == info['gmin'] or True  # per-core gmins differ!

    # gmin windows are baked into the program; they differ per core, but the
    # program is shared SPMD. Use per-core max window start? No: windows must
    # be identical across cores. Instead bake core-0's?  -> handled below.
    key = (tuple(bounds), tuple(tuple(info['gmin']) for info in cores))
    if _cache.get('key') != key:
        # SPMD: all cores share one program, so gmin windows must be common.
        # Use per-chunk windows covering all cores? Not possible if they
        # differ. We instead pass sel shifted so that window starts are the
        # SAME for all cores: chunk ci uses window start gmin_common[ci] =
        # min over cores, and sel columns absorb the difference as long as
        # slot - gmin_common < 8 for every core. Verify and widen if needed.
        gmin_common = []
        ok = True
        for ci in range(NPC):
            lo = min(info['gmin'][ci] for info in cores)
            hi = max(info['gmin'][ci] + 7 for info in cores)
            if hi - lo >= 8:
                ok = False
            gmin_common.append(lo)
        if not ok:
            raise RuntimeError("per-core gmin windows diverge; widen window")
        _cache['nc'] = _build_program(bounds, gmin_common)
        _cache['key'] = key
        _cache['gmin_common'] = gmin_common
    nc = _cache['nc']
    gmin_common = _cache['gmin_common']

    # rebuild sel against common windows
    for c, info in enumerate(cores):
        sel = np.zeros((128, NPC * 8), np.float32)
        size_c = info['size']
        # recompute from slots
        # (cheap: reuse original sel shifted by gmin difference)
        shift_ok = True
        src_sel = info['sel']
        for ci in range(NPC):
            d = info['gmin'][ci] - gmin_common[ci]
            assert 0 <= d < 8
            if d == 0:
                sel[:, ci * 8:(ci + 1) * 8] = src_sel[:, ci * 8:(ci + 1) * 8]
            else:
                sel[:, ci * 8 + d:(ci + 1) * 8] = \
                    src_sel[:, ci * 8:(ci + 1) * 8 - d]
                assert not src_sel[:, (ci + 1) * 8 - d:(ci + 1) * 8].any()
        in_maps[c]['sel'] = to_bf16(sel)

    res = run_bass_kernel_spmd(nc, in_maps, core_ids=list(range(NC)),
                               trace=bool(os.environ.get("KERNEL_TRACE")))
    _cache['last_results'] = res

    out = np.zeros((N_GRAPHS, 1), np.float32)
    for c, info in enumerate(cores):
        vals = np.asarray(res.results[c]["outg"], np.float32).reshape(-1)
        out[info['g_lo']:info['g_lo'] + info['g_cnt'], 0] = \
            vals[:info['g_cnt']]
    return out
